# revision 16
# baseline (speedup 1.0000x reference)
"""Differential cross-attention Trainium2 kernel (8 NeuronCores).

Sharding: 8 cores = (batch b = c//2) x (query half = c%2). Each core
computes all 8 heads for its 512 queries against all 1024 keys of its
batch: projections, RPE-biased scores, softmax (exp without max
subtraction; |scores| <= ~2), differential combine, output projection.

Layouts (per core, fp32):
  xq_T [512d, 512q], xkv_T [512d, 1024k] host-transposed.
  Q_T/K_T [d_out, n] via PE (weights as lhsT [di, do]); V [k, dv].
  Scores transposed S_T[k, q] per (head, k-tile): RPE bias preloaded
  into the PSUM accumulator so the bias add is free; U = exp(S_T) on
  ACT feeds PV matmuls directly as lhsT (k on partitions). Softmax
  denominators come from N=1 ones-column matmuls. Differential
  combine is folded into per-q scalars:
    x1 = (1+alpha)/S1 * U1V1 - alpha*lam/S2 * U2V1,  x2 = U2V2/S2.
"""
import sys
sys.path.insert(0, "/opt/trn_rl_repo")
import numpy as np

DIM = 512
H = 8
HD = 64
NQC = 512
NKV = 1024
MAX_DIST = 128
LAMBDA_INIT = 0.8
N_CORES = 8
SCALE = HD ** -0.5

_COMPILED = {}


def _build(reps=1):
    import concourse.bacc as bacc
    import concourse.mybir as mybir
    from concourse.tile import TileContext
    from concourse.masks import make_identity

    f32 = mybir.dt.float32
    bf16 = mybir.dt.bfloat16
    nc = bacc.Bacc("TRN2", target_bir_lowering=False, debug=False,
                   num_devices=N_CORES)

    xq_T = nc.dram_tensor("xq_T", [DIM, NQC], f32, kind="ExternalInput")
    xkv_T = nc.dram_tensor("xkv_T", [DIM, NKV], f32, kind="ExternalInput")
    wq = nc.dram_tensor("wq", [DIM, DIM], f32, kind="ExternalInput")
    wk = nc.dram_tensor("wk", [DIM, DIM], f32, kind="ExternalInput")
    wv = nc.dram_tensor("wv", [DIM, DIM], f32, kind="ExternalInput")
    wp = nc.dram_tensor("wp", [DIM, DIM], f32, kind="ExternalInput")
    bq = nc.dram_tensor("bq", [128, 4], f32, kind="ExternalInput")
    bk = nc.dram_tensor("bk", [128, 4], f32, kind="ExternalInput")
    bv = nc.dram_tensor("bv", [128, DIM], f32, kind="ExternalInput")
    bp = nc.dram_tensor("bp", [128, 4], f32, kind="ExternalInput")
    alpha = nc.dram_tensor("alpha", [128, 4], f32, kind="ExternalInput")
    lam_in = nc.dram_tensor("lam", [128, 4], f32, kind="ExternalInput")
    biasT = nc.dram_tensor("biasT", [H, 8, 128, NQC], bf16, kind="ExternalInput")
    out_T = nc.dram_tensor("out_T", [DIM, NQC], f32, kind="ExternalOutput")

    with TileContext(nc) as tc:
      for _rep in range(reps):
        with (
            tc.tile_pool(name="const", bufs=1) as cpool,
            tc.tile_pool(name="work", bufs=1) as wpool,
            tc.tile_pool(name="stream", bufs=6) as spool,
            tc.tile_pool(name="psum", bufs=2, space="PSUM") as ppool,
            tc.tile_pool(name="psc", bufs=4, space="PSUM") as scpool,
            tc.tile_pool(name="psacc", bufs=2, space="PSUM") as papool,
        ):
            wq_t = cpool.tile([128, 4, DIM], f32, tag="wq")
            wk_t = cpool.tile([128, 4, DIM], f32, tag="wk")
            wv_t = cpool.tile([128, 4, DIM], f32, tag="wv")
            wp_t = cpool.tile([128, 4, DIM], f32, tag="wp")
            for w_t, w in ((wq_t, wq), (wk_t, wk), (wv_t, wv), (wp_t, wp)):
                nc.sync.dma_start(out=w_t[:], in_=w[:].rearrange("(c p) o -> p c o", p=128))
            bq_t = cpool.tile([128, 4], f32, tag="bq")
            bk_t = cpool.tile([128, 4], f32, tag="bk")
            bv_t = cpool.tile([128, DIM], f32, tag="bv")
            bp_t = cpool.tile([128, 4], f32, tag="bp")
            al_t = cpool.tile([128, 4], f32, tag="al")
            lam_t = cpool.tile([128, 4], f32, tag="lam")
            for t, src in ((bq_t, bq), (bk_t, bk), (bv_t, bv), (bp_t, bp),
                           (al_t, alpha), (lam_t, lam_in)):
                nc.sync.dma_start(out=t[:], in_=src[:])
            ones_col = cpool.tile([128, 1], f32, tag="ones")
            nc.vector.memset(ones_col[:], 1.0)
            al1_t = cpool.tile([128, 4], f32, tag="al1")
            nc.vector.tensor_scalar(out=al1_t[:], in0=al_t[:], scalar1=1.0,
                                    scalar2=None, op0=mybir.AluOpType.add)
            alam_t = cpool.tile([128, 4, 4], f32, tag="alam")
            ident = cpool.tile([128, 128], f32, tag="ident")
            make_identity(nc, ident[:])
            ident_h = cpool.tile([128, 128], bf16, tag="identh")
            nc.vector.tensor_copy(out=ident_h[:], in_=ident[:])

            xq_t = wpool.tile([128, 4, NQC], f32, tag="xq")
            xkv_t = wpool.tile([128, 4, NKV], f32, tag="xkv")
            nc.sync.dma_start(out=xq_t[:], in_=xq_T[:].rearrange("(c p) n -> p c n", p=128))
            nc.sync.dma_start(out=xkv_t[:], in_=xkv_T[:].rearrange("(c p) n -> p c n", p=128))

            q_sb = cpool.tile([128, 4, NQC], f32, tag="qsb")
            k_sb = cpool.tile([128, 4, NKV], f32, tag="ksb")
            v_sb = cpool.tile([128, 8, DIM], f32, tag="vsb")
            for t in range(4):
                ps = ppool.tile([128, NQC], f32, tag="proj")
                for c in range(4):
                    nc.tensor.matmul(ps[:], lhsT=wq_t[:, c, 128 * t:128 * (t + 1)],
                                     rhs=xq_t[:, c, :], start=(c == 0), stop=(c == 3))
                nc.vector.tensor_scalar(out=q_sb[:, t, :], in0=ps[:],
                                        scalar1=bq_t[:, t:t + 1], scalar2=None,
                                        op0=mybir.AluOpType.add)
            for t in range(4):
                for kh in range(2):
                    ps = ppool.tile([128, NQC], f32, tag="proj")
                    for c in range(4):
                        nc.tensor.matmul(
                            ps[:], lhsT=wk_t[:, c, 128 * t:128 * (t + 1)],
                            rhs=xkv_t[:, c, 512 * kh:512 * (kh + 1)],
                            start=(c == 0), stop=(c == 3))
                    nc.vector.tensor_scalar(out=k_sb[:, t, 512 * kh:512 * (kh + 1)],
                                            in0=ps[:], scalar1=bk_t[:, t:t + 1],
                                            scalar2=None, op0=mybir.AluOpType.add)
            for m in range(8):
                ps = ppool.tile([128, NQC], f32, tag="proj")
                for c in range(4):
                    nc.tensor.matmul(ps[:], lhsT=xkv_t[:, c, 128 * m:128 * (m + 1)],
                                     rhs=wv_t[:, c, :], start=(c == 0), stop=(c == 3))
                nc.vector.tensor_tensor(out=v_sb[:, m, :], in0=ps[:], in1=bv_t[:],
                                        op=mybir.AluOpType.add)

            # extended V tiles: per (pair, m): ve1 = [v_h1 | 1] (65),
            # ve2 = [v_h2 | v_h1 | 1] (129)
            ve1 = cpool.tile([128, 4, 8, 65], bf16, tag="ve1")
            ve2 = cpool.tile([128, 4, 8, 129], bf16, tag="ve2")
            for hp in range(4):
                h1, h2 = hp, hp + 4
                for m in range(8):
                    nc.vector.tensor_copy(out=ve1[:, hp, m, 0:64],
                                          in_=v_sb[:, m, 64 * h1:64 * (h1 + 1)])
                    nc.vector.memset(ve1[:, hp, m, 64:65], 1.0)
                    nc.vector.tensor_copy(out=ve2[:, hp, m, 0:64],
                                          in_=v_sb[:, m, 64 * h2:64 * (h2 + 1)])
                    nc.vector.tensor_copy(out=ve2[:, hp, m, 64:128],
                                          in_=v_sb[:, m, 64 * h1:64 * (h1 + 1)])
                    nc.vector.memset(ve2[:, hp, m, 128:129], 1.0)

            xcat = wpool.tile([128, 4, DIM], f32, tag="xcat")
            for hp in range(4):
                h1, h2 = hp, hp + 4
                t1, r1 = divmod(h1, 2)
                t2, r2 = divmod(h2, 2)
                nc.vector.tensor_scalar(out=alam_t[:, hp, :], in0=al_t[:],
                                        scalar1=lam_t[:, hp:hp + 1], scalar2=None,
                                        op0=mybir.AluOpType.mult)
                pvbanks = []
                for _qb in range(2):
                    pv_bank = papool.tile([128, 388], f32, tag="pvacc")
                    pvbanks.append(pv_bank)
                pvs = [pvbanks[qt // 2][:, 194 * (qt % 2):194 * (qt % 2 + 1)]
                       for qt in range(4)]
                for m in range(8):
                    u_tiles = []
                    for (h, tt, rr) in ((h1, t1, r1), (h2, t2, r2)):
                        ss = scpool.tile([128, NQC], f32, tag="scores")
                        bt = spool.tile([128, NQC], bf16, tag="biasin")
                        nc.sync.dma_start(out=bt[:], in_=biasT[h, m, :, :])
                        nc.tensor.matmul(ss[:], lhsT=ident_h[:], rhs=bt[:],
                                         start=True, stop=False)
                        nc.tensor.matmul(
                            ss[:],
                            lhsT=k_sb[64 * rr:64 * (rr + 1), tt, 128 * m:128 * (m + 1)],
                            rhs=q_sb[64 * rr:64 * (rr + 1), tt, :],
                            start=False, stop=True)
                        u = spool.tile([128, NQC], bf16, tag="u")
                        nc.scalar.activation(u[:], ss[:],
                                             mybir.ActivationFunctionType.Exp)
                        u_tiles.append(u)
                    u1, u2 = u_tiles
                    first, last = (m == 0), (m == 7)
                    for qt in range(4):
                        q0 = 128 * qt
                        pv = pvs[qt]
                        st = first and (qt % 2 == 0)
                        sp = last and (qt % 2 == 1)
                        nc.tensor.matmul(pv[:, 0:65], lhsT=u1[:, q0:q0 + 128],
                                         rhs=ve1[:, hp, m, :], start=st, stop=False)
                        nc.tensor.matmul(pv[:, 65:194], lhsT=u2[:, q0:q0 + 128],
                                         rhs=ve2[:, hp, m, :], start=False, stop=sp)
                for qt in range(4):
                    pv = pvs[qt]
                    rs1 = spool.tile([128, 1], f32, tag="rs1")
                    rs2 = spool.tile([128, 1], f32, tag="rs2")
                    nc.vector.reciprocal(rs1[:], pv[:, 64:65])
                    nc.vector.reciprocal(rs2[:], pv[:, 193:194])
                    tmp1 = spool.tile([128, 64], f32, tag="tmp1")
                    nc.vector.tensor_scalar(out=tmp1[:], in0=pv[:, 0:64],
                                            scalar1=rs1[:],
                                            scalar2=al1_t[:, qt:qt + 1],
                                            op0=mybir.AluOpType.mult,
                                            op1=mybir.AluOpType.mult)
                    tmp2 = spool.tile([128, 64], f32, tag="tmp2")
                    nc.vector.tensor_scalar(out=tmp2[:], in0=pv[:, 129:193],
                                            scalar1=rs2[:],
                                            scalar2=alam_t[:, hp, qt:qt + 1],
                                            op0=mybir.AluOpType.mult,
                                            op1=mybir.AluOpType.mult)
                    nc.vector.tensor_tensor(out=xcat[:, qt, 64 * h1:64 * (h1 + 1)],
                                            in0=tmp1[:], in1=tmp2[:],
                                            op=mybir.AluOpType.subtract)
                    nc.vector.tensor_scalar(out=xcat[:, qt, 64 * h2:64 * (h2 + 1)],
                                            in0=pv[:, 65:129],
                                            scalar1=rs2[:], scalar2=None,
                                            op0=mybir.AluOpType.mult)

            xcat_T = wpool.tile([128, 4, NQC], f32, tag="xcatT")
            for dit in range(4):
                for qt in range(4):
                    pst = ppool.tile([128, 128], f32, tag="proj")
                    nc.tensor.transpose(out=pst[:],
                                        in_=xcat[:, qt, 128 * dit:128 * (dit + 1)],
                                        identity=ident[:])
                    nc.vector.tensor_copy(out=xcat_T[:, dit, 128 * qt:128 * (qt + 1)],
                                          in_=pst[:])
            for t in range(4):
                ps = ppool.tile([128, NQC], f32, tag="proj")
                for c in range(4):
                    nc.tensor.matmul(ps[:], lhsT=wp_t[:, c, 128 * t:128 * (t + 1)],
                                     rhs=xcat_T[:, c, :], start=(c == 0), stop=(c == 3))
                ot = wpool.tile([128, NQC], f32, tag="otile")
                nc.vector.tensor_scalar(out=ot[:], in0=ps[:],
                                        scalar1=bp_t[:, t:t + 1], scalar2=None,
                                        op0=mybir.AluOpType.add)
                nc.sync.dma_start(
                    out=out_T[:].rearrange("(c p) n -> p c n", p=128)[:, t, :],
                    in_=ot[:])
    nc.compile()
    return nc


def _get_kernel(reps=1):
    key = f"k{reps}"
    if key not in _COMPILED:
        _COMPILED[key] = _build(reps)
    return _COMPILED[key]


def _to_bf16(a):
    import jax.numpy as jnp
    return np.asarray(jnp.asarray(a, dtype=jnp.bfloat16))


def _prep_inputs(x_q, x_kv, coords_q, coords_k, alpha_map,
                 Wq, bq, Wk, bk, Wv, bv,
                 lambda_q1, lambda_k1, lambda_q2, lambda_k2,
                 rpe_table, Wp, bp):
    x_q = np.asarray(x_q, dtype=np.float32)
    x_kv = np.asarray(x_kv, dtype=np.float32)
    coords_q = np.asarray(coords_q)
    coords_k = np.asarray(coords_k)
    alpha_map = np.asarray(alpha_map, dtype=np.float32)
    rpe = np.asarray(rpe_table, dtype=np.float32)

    lam1 = np.exp(np.sum(np.asarray(lambda_q1) * np.asarray(lambda_k1), axis=-1))
    lam2 = np.exp(np.sum(np.asarray(lambda_q2) * np.asarray(lambda_k2), axis=-1))
    lam = (lam1 - lam2 + LAMBDA_INIT).astype(np.float32)
    lam_rep = np.ascontiguousarray(np.tile(lam[None, :], (128, 1)))

    wq_l = np.ascontiguousarray(np.asarray(Wq, dtype=np.float32).T * SCALE)
    wk_l = np.ascontiguousarray(np.asarray(Wk, dtype=np.float32).T)
    wv_l = np.ascontiguousarray(np.asarray(Wv, dtype=np.float32).T)
    wp_l = np.ascontiguousarray(np.asarray(Wp, dtype=np.float32).T)
    bq_l = np.ascontiguousarray((np.asarray(bq, dtype=np.float32) * SCALE).reshape(4, 128).T)
    bk_l = np.ascontiguousarray(np.asarray(bk, dtype=np.float32).reshape(4, 128).T)
    bv_l = np.ascontiguousarray(np.tile(np.asarray(bv, dtype=np.float32)[None, :], (128, 1)))
    bp_l = np.ascontiguousarray(np.asarray(bp, dtype=np.float32).reshape(4, 128).T)

    in_maps = []
    for c in range(N_CORES):
        b, qh = divmod(c, 2)
        qsl = slice(qh * NQC, (qh + 1) * NQC)
        cq = coords_q[b, qsl]
        ck = coords_k[b]
        rel = cq[:, None, :] - ck[None, :, :] + MAX_DIST
        rel = np.clip(rel, 0, 2 * MAX_DIST)
        idx = rel[..., 0] * (2 * MAX_DIST + 1) + rel[..., 1]
        bias = rpe[idx]                                        # [512q, 1024k, 8]
        biasT = np.ascontiguousarray(bias.transpose(2, 1, 0))  # [8, 1024k, 512q]
        in_maps.append({
            "xq_T": np.ascontiguousarray(x_q[b, qsl].T),
            "xkv_T": np.ascontiguousarray(x_kv[b].T),
            "wq": wq_l, "wk": wk_l, "wv": wv_l, "wp": wp_l,
            "bq": bq_l, "bk": bk_l, "bv": bv_l, "bp": bp_l,
            "alpha": np.ascontiguousarray(alpha_map[b, qsl, 0].reshape(4, 128).T),
            "lam": lam_rep,
            "biasT": _to_bf16(biasT.reshape(H, 8, 128, NQC)),
        })
    return in_maps


def kernel(x_q, x_kv, coords_q, coords_k, alpha_map,
           Wq, bq, Wk, bk, Wv, bv,
           lambda_q1, lambda_k1, lambda_q2, lambda_k2,
           rpe_table, Wp, bp):
    from concourse.bass_utils import run_bass_kernel_spmd

    nc = _get_kernel()
    in_maps = _prep_inputs(x_q, x_kv, coords_q, coords_k, alpha_map,
                           Wq, bq, Wk, bk, Wv, bv,
                           lambda_q1, lambda_k1, lambda_q2, lambda_k2,
                           rpe_table, Wp, bp)
    res = run_bass_kernel_spmd(nc, in_maps, list(range(N_CORES)))
    B = np.asarray(x_q).shape[0]
    out = np.zeros((B, 2 * NQC, DIM), dtype=np.float32)
    for c in range(N_CORES):
        b, qh = divmod(c, 2)
        out[b, qh * NQC:(qh + 1) * NQC] = res.results[c]["out_T"].T
    return out
